# revision 11
# baseline (speedup 1.0000x reference)
"""Trainium2 Bass kernel for additive (coverage) attention.

Reference computation (per example b):
    e      = tanh(enc @ W_h + b_h + ds @ W_s + b_s + cov @ W_c + b_c)   [S, H]
    scores = e @ v                                                       [S, 1]
    attn   = softmax(scores, axis=0)
    h_star = attn.T @ enc                                                [1, H]
    covnew = cov + attn                                                  [S, H]

Shapes: B=32, S=4096, H=512. Sharded data-parallel over batch across 8 cores
(4 examples per core); weights replicated. Small weight tensors are
layout-prepped on host (rearrange to partition-major, bf16 cast, bias sum).

Per-core strategy (memory-bound target, ~100MB HBM traffic/core):
  - enc/cov stream in fp32 s-major (natural layout, efficient DMA).
  - GpSimd casts them to bf16; DMA-XBAR transposes (SBUF->SBUF) produce
    feature-major bf16 tiles for the TensorE contraction over features.
  - X.T accumulates in PSUM f-major; ScalarE applies tanh with the combined
    bias (b_h+b_s+b_c+ds@W_s) folded in as a per-partition bias, writing bf16.
  - scores = v . tanh(X) via TensorE (v as stationary), softmax on-chip.
  - h_star via TensorE (attn columns stationary, bf16 s-major enc moving).
  - covnew = fp32 cov (kept resident) + attn, exact in fp32.
"""

import ml_dtypes
import numpy as np

import concourse.bass as bass
import concourse.mybir as mybir
from concourse import bacc
from concourse.bass_utils import run_bass_kernel_spmd
from concourse.masks import make_identity
from concourse.tile import TileContext

F32 = mybir.dt.float32
BF16 = mybir.dt.bfloat16
AF = mybir.ActivationFunctionType
ALU = mybir.AluOpType

B, S, H = 32, 4096, 512
N_CORES = 8
BPC = B // N_CORES  # examples per core

P = 128          # partitions
C = H // P       # feature chunks (4)
CH = 512         # s-chunk (matmul moving free dim)
TPC = CH // P    # s-tiles per chunk (4)


def build_nc(bpc: int, s_len: int) -> bass.Bass:
    NCH = s_len // CH    # chunks per example
    NT = s_len // P      # s-tiles per example

    nc = bacc.Bacc(None, target_bir_lowering=False)

    # Large inputs (per-core batch shard), natural layout.
    enc_d = nc.dram_tensor("encoder_outputs", [bpc, s_len, H], F32, kind="ExternalInput")
    cov_d = nc.dram_tensor("coverage_vector", [bpc, s_len, H], F32, kind="ExternalInput")
    # Host-prepped small tensors, partition-major layouts.
    dsT_d = nc.dram_tensor("dsT", [P, C, bpc], F32, kind="ExternalInput")
    Whb_d = nc.dram_tensor("Whb", [P, C, H], BF16, kind="ExternalInput")
    Wcb_d = nc.dram_tensor("Wcb", [P, C, H], BF16, kind="ExternalInput")
    Wsr_d = nc.dram_tensor("Wsr", [P, C, H], F32, kind="ExternalInput")
    bsum_d = nc.dram_tensor("bsumT", [P, C], F32, kind="ExternalInput")
    vb_d = nc.dram_tensor("vTb", [P, C], BF16, kind="ExternalInput")

    hst_d = nc.dram_tensor("h_star", [bpc, 1, H], F32, kind="ExternalOutput")
    attn_d = nc.dram_tensor("attn_dist", [bpc, s_len, 1], F32, kind="ExternalOutput")
    covn_d = nc.dram_tensor("coverage_new", [bpc, s_len, H], F32, kind="ExternalOutput")

    with TileContext(nc) as tc:
        with (
            tc.tile_pool(name="const", bufs=1) as const,
            tc.tile_pool(name="encf", bufs=3) as encf_pool,
            tc.tile_pool(name="covf", bufs=NCH + 1) as covf_pool,
            tc.tile_pool(name="encb", bufs=NCH + 1) as encb_pool,
            tc.tile_pool(name="covb", bufs=2) as covb_pool,
            tc.tile_pool(name="tpose", bufs=2) as tpose_pool,
            tc.tile_pool(name="tanh", bufs=3) as tanh_pool,
            tc.tile_pool(name="ex", bufs=2) as ex_pool,
            tc.tile_pool(name="psx", bufs=3, space="PSUM") as psx_pool,
            tc.tile_pool(name="pssc", bufs=2, space="PSUM") as pssc_pool,
            tc.tile_pool(name="pssm", bufs=2, space="PSUM") as pssm_pool,
            tc.tile_pool(name="pshs", bufs=1, space="PSUM") as pshs_pool,
        ):
            # ---------- constants ----------
            ident = const.tile([P, P], F32, tag="ident")
            make_identity(nc, ident)
            ones_row = const.tile([1, P], F32, tag="ones_row")
            nc.vector.memset(ones_row, 1.0)
            one11 = const.tile([1, 1], F32, tag="one11")
            nc.vector.memset(one11, 1.0)
            ones_col = const.tile([P, 1], F32, tag="ones_col")
            nc.vector.memset(ones_col, 1.0)

            Whb = const.tile([P, C, H], BF16, tag="Whb")
            nc.sync.dma_start(Whb, Whb_d[:])
            Wcb = const.tile([P, C, H], BF16, tag="Wcb")
            nc.sync.dma_start(Wcb, Wcb_d[:])
            Wsr = const.tile([P, C, H], F32, tag="Wsr")
            nc.sync.dma_start(Wsr, Wsr_d[:])
            bsumT = const.tile([P, C], F32, tag="bsumT")
            nc.sync.dma_start(bsumT, bsum_d[:])
            vTb = const.tile([P, C], BF16, tag="vTb")
            nc.sync.dma_start(vTb, vb_d[:])
            dsT = const.tile([P, C, bpc], F32, tag="dsT")
            nc.sync.dma_start(dsT, dsT_d[:])

            # c_b for all examples: [P, C(fout chunk), bpc]
            cbT = const.tile([P, C, bpc], F32, tag="cbT")
            for m in range(C):
                ps = pssm_pool.tile([P, bpc], F32, tag="small")
                for c in range(C):
                    nc.tensor.matmul(
                        ps,
                        lhsT=Wsr[:, c, m * P:(m + 1) * P],
                        rhs=dsT[:, c, :],
                        start=(c == 0),
                        stop=(c == C - 1),
                    )
                nc.vector.tensor_tensor(
                    cbT[:, m, :], ps,
                    bsumT[:, m:m + 1].to_broadcast((P, bpc)),
                    ALU.add,
                )

            for ex in range(bpc):
                # scores in s-major layout: partition = s % P, col = s // P
                scsm = ex_pool.tile([P, NT], F32, tag="scsm")
                enc_bf = []
                cov_f = []

                # ---------- phase 1: X = tanh(...), scores ----------
                for k in range(NCH):
                    sl = slice(k * CH, (k + 1) * CH)
                    e_f = encf_pool.tile([P, TPC, H], F32, tag="encf")
                    nc.sync.dma_start(e_f, enc_d[ex, sl, :].rearrange("(t p) f -> p t f", p=P))
                    c_f = covf_pool.tile([P, TPC, H], F32, tag="covf")
                    nc.sync.dma_start(c_f, cov_d[ex, sl, :].rearrange("(t p) f -> p t f", p=P))
                    cov_f.append(c_f)

                    e_b = encb_pool.tile([P, TPC, H], BF16, tag="encb")
                    nc.gpsimd.tensor_copy(e_b, e_f)
                    enc_bf.append(e_b)
                    c_b = covb_pool.tile([P, TPC, H], BF16, tag="covb")
                    nc.gpsimd.tensor_copy(c_b, c_f)

                    # One XBAR transpose per matrix-chunk: in [128 s, (t f)] ->
                    # out[p, q, s] = in[s, q*128+p], i.e. q = t*C + c.
                    encT_r = tpose_pool.tile([P, C * TPC, P], BF16, tag="encT")
                    covT_r = tpose_pool.tile([P, C * TPC, P], BF16, tag="covT")
                    nc.scalar.dma_start_transpose(encT_r, e_b.rearrange("p t f -> p (t f)"))
                    nc.scalar.dma_start_transpose(covT_r, c_b.rearrange("p t f -> p (t f)"))
                    # moving operand for feature chunk c: [:, c] is [128, t, s] = s-chunk
                    encT = encT_r.rearrange("p (t c) s -> p c t s", c=C)
                    covT = covT_r.rearrange("p (t c) s -> p c t s", c=C)

                    sc_ps = pssc_pool.tile([1, CH], F32, tag="scps")
                    for m in range(C):
                        x_ps = psx_pool.tile([P, CH], F32, tag="xps")
                        for c in range(C):
                            nc.tensor.matmul(
                                x_ps,
                                lhsT=Whb[:, c, m * P:(m + 1) * P],
                                rhs=encT[:, c, :],
                                start=(c == 0),
                                stop=False,
                            )
                        for c in range(C):
                            nc.tensor.matmul(
                                x_ps,
                                lhsT=Wcb[:, c, m * P:(m + 1) * P],
                                rhs=covT[:, c, :],
                                start=False,
                                stop=(c == C - 1),
                            )
                        th = tanh_pool.tile([P, CH], BF16, tag="tanh")
                        nc.scalar.activation(th, x_ps, AF.Tanh, bias=cbT[:, m, ex:ex + 1])
                        nc.tensor.matmul(
                            sc_ps,
                            lhsT=vTb[:, m:m + 1],
                            rhs=th,
                            start=(m == 0),
                            stop=(m == C - 1),
                        )
                    sc_sb = tanh_pool.tile([1, CH], F32, tag="sc_sb")
                    nc.vector.tensor_copy(sc_sb, sc_ps)
                    # transpose chunk scores into s-major columns of scsm
                    sct_ps = pssm_pool.tile([P, TPC], F32, tag="small")
                    for t in range(TPC):
                        nc.tensor.matmul(
                            sct_ps[:, t:t + 1],
                            lhsT=sc_sb[:, t * P:(t + 1) * P],
                            rhs=one11,
                            start=True,
                            stop=True,
                        )
                    nc.vector.tensor_copy(scsm[:, k * TPC:(k + 1) * TPC], sct_ps)

                # ---------- softmax over all S ----------
                m1 = ex_pool.tile([P, 1], F32, tag="m1")
                nc.vector.tensor_reduce(m1, scsm, mybir.AxisListType.X, ALU.max)
                mT_ps = pssm_pool.tile([1, P], F32, tag="small")
                nc.tensor.transpose(mT_ps, m1, ident)
                mT = ex_pool.tile([1, P], F32, tag="mT")
                nc.vector.tensor_copy(mT, mT_ps)
                m0 = ex_pool.tile([1, 1], F32, tag="m0")
                nc.vector.tensor_reduce(m0, mT, mybir.AxisListType.X, ALU.max)

                # broadcast -max to [P, 1]
                nm_ps = pssm_pool.tile([P, 1], F32, tag="small")
                nc.tensor.matmul(nm_ps, lhsT=ones_row, rhs=m0, start=True, stop=True)
                negm = ex_pool.tile([P, 1], F32, tag="negm")
                nc.vector.tensor_scalar_mul(negm, nm_ps, -1.0)

                esm = ex_pool.tile([P, NT], F32, tag="esm")
                rowsum = ex_pool.tile([P, 1], F32, tag="rowsum")
                nc.scalar.activation(esm, scsm, AF.Exp, bias=negm, accum_out=rowsum)

                tot_ps = pssm_pool.tile([1, 1], F32, tag="small")
                nc.tensor.matmul(tot_ps, lhsT=rowsum, rhs=ones_col, start=True, stop=True)
                tot = ex_pool.tile([1, 1], F32, tag="tot")
                nc.vector.tensor_copy(tot, tot_ps)
                rtot = ex_pool.tile([1, 1], F32, tag="rtot")
                nc.vector.reciprocal(rtot, tot)
                rb_ps = pssm_pool.tile([P, 1], F32, tag="small")
                nc.tensor.matmul(rb_ps, lhsT=ones_row, rhs=rtot, start=True, stop=True)
                rbc = ex_pool.tile([P, 1], F32, tag="rbc")
                nc.vector.tensor_copy(rbc, rb_ps)

                attn_sm = ex_pool.tile([P, NT], F32, tag="attn_sm")
                nc.vector.tensor_scalar_mul(attn_sm, esm, rbc)
                attn_bf = ex_pool.tile([P, NT], BF16, tag="attn_bf")
                nc.vector.tensor_copy(attn_bf, attn_sm)

                # attn output, s-contiguous [NT, P]
                at_ps = pssm_pool.tile([NT, P], F32, tag="small")
                nc.tensor.transpose(at_ps, attn_sm, ident)
                at_sb = ex_pool.tile([NT, P], F32, tag="at_sb")
                nc.vector.tensor_copy(at_sb, at_ps)
                nc.sync.dma_start(attn_d[ex, :, 0].rearrange("(t p) -> t p", p=P), at_sb)

                # ---------- phase 2: h_star, coverage_new ----------
                hs_ps = pshs_pool.tile([1, H], F32, tag="hs")
                for k in range(NCH):
                    sl = slice(k * CH, (k + 1) * CH)
                    for t in range(TPC):
                        kt = k * TPC + t
                        nc.tensor.matmul(
                            hs_ps,
                            lhsT=attn_bf[:, kt:kt + 1],
                            rhs=enc_bf[k][:, t, :],
                            start=(kt == 0),
                            stop=(kt == NT - 1),
                        )
                    for t in range(TPC):
                        kt = k * TPC + t
                        nc.vector.tensor_scalar_add(
                            cov_f[k][:, t, :], cov_f[k][:, t, :], attn_sm[:, kt:kt + 1]
                        )
                    nc.scalar.dma_start(
                        covn_d[ex, sl, :].rearrange("(t p) f -> p t f", p=P), cov_f[k]
                    )
                hst_sb = ex_pool.tile([1, H], F32, tag="hst_sb")
                nc.vector.tensor_copy(hst_sb, hs_ps)
                nc.sync.dma_start(hst_d[ex:ex + 1, 0, :], hst_sb)

    nc.compile()
    return nc


_NC_CACHE = {}


def _get_nc(bpc, s_len):
    key = (bpc, s_len)
    if key not in _NC_CACHE:
        _NC_CACHE[key] = build_nc(bpc, s_len)
    return _NC_CACHE[key]


def _prep_pmajor(W):
    """[H, H] -> [P, C, H] with partition = f_in % P, c = f_in // P."""
    return np.ascontiguousarray(W.reshape(C, P, H).transpose(1, 0, 2))


def kernel(decoder_state, encoder_outputs, coverage_vector,
           W_h, b_h, W_s, b_s, W_c, b_c, v, _trace=False, _s_len=S, _n_cores=N_CORES):
    decoder_state = np.asarray(decoder_state, dtype=np.float32)
    encoder_outputs = np.ascontiguousarray(np.asarray(encoder_outputs, dtype=np.float32))
    coverage_vector = np.ascontiguousarray(np.asarray(coverage_vector, dtype=np.float32))
    W_h = np.asarray(W_h, dtype=np.float32)
    W_s = np.asarray(W_s, dtype=np.float32)
    W_c = np.asarray(W_c, dtype=np.float32)
    b_sum = (np.asarray(b_h, dtype=np.float32) + np.asarray(b_s, dtype=np.float32)
             + np.asarray(b_c, dtype=np.float32))
    v = np.asarray(v, dtype=np.float32)

    n_cores = _n_cores
    batch = decoder_state.shape[0]
    bpc = batch // n_cores
    nc = _get_nc(bpc, _s_len)

    shared = {
        "Whb": _prep_pmajor(W_h).astype(ml_dtypes.bfloat16),
        "Wcb": _prep_pmajor(W_c).astype(ml_dtypes.bfloat16),
        "Wsr": _prep_pmajor(W_s),
        "bsumT": np.ascontiguousarray(b_sum.reshape(C, P).T),
        "vTb": np.ascontiguousarray(v.reshape(C, P).T).astype(ml_dtypes.bfloat16),
    }

    in_maps = []
    for i in range(n_cores):
        sl = slice(i * bpc, (i + 1) * bpc)
        # dsT: [P, C, bpc] feature-major decoder state for this core's shard
        dsT = np.ascontiguousarray(
            decoder_state[sl, 0, :].T.reshape(C, P, bpc).transpose(1, 0, 2)
        )
        m = {
            "encoder_outputs": encoder_outputs[sl],
            "coverage_vector": coverage_vector[sl],
            "dsT": dsT,
        }
        m.update(shared)
        in_maps.append(m)

    res = run_bass_kernel_spmd(nc, in_maps, list(range(n_cores)), trace=_trace)
    h_star = np.concatenate([r["h_star"] for r in res.results], axis=0)
    attn = np.concatenate([r["attn_dist"] for r in res.results], axis=0)
    covnew = np.concatenate([r["coverage_new"] for r in res.results], axis=0)
    kernel.last_exec_time_ns = res.exec_time_ns
    kernel.last_results = res
    return h_star, attn, covnew


# revision 13
# speedup vs baseline: 19.3782x; 19.3782x over previous
"""Trainium2 Bass kernel for additive (coverage) attention.

Reference computation (per example b):
    e      = tanh(enc @ W_h + b_h + ds @ W_s + b_s + cov @ W_c + b_c)   [S, H]
    scores = e @ v                                                       [S, 1]
    attn   = softmax(scores, axis=0)
    h_star = attn.T @ enc                                                [1, H]
    covnew = cov + attn                                                  [S, H]

Shapes: B=32, S=4096, H=512. Sharded data-parallel over batch across 8 cores
(4 examples per core); weights replicated. Small weight tensors are
layout-prepped on host (rearrange to partition-major, bf16 cast, bias sum).

Per-core strategy (memory-bound target, ~100MB HBM traffic/core):
  - enc/cov stream in fp32 s-major (natural layout, efficient DMA).
  - GpSimd casts them to bf16; DMA-XBAR transposes (SBUF->SBUF) produce
    feature-major bf16 tiles for the TensorE contraction over features.
  - X.T accumulates in PSUM f-major; ScalarE applies tanh with the combined
    bias (b_h+b_s+b_c+ds@W_s) folded in as a per-partition bias, writing bf16.
  - scores = v . tanh(X) via TensorE (v as stationary), softmax on-chip.
  - h_star via TensorE (attn columns stationary, bf16 s-major enc moving).
  - covnew = fp32 cov (kept resident) + attn, exact in fp32.
"""

import ml_dtypes
import numpy as np

import concourse.bass as bass
import concourse.mybir as mybir
from concourse import bacc
from concourse.bass_utils import run_bass_kernel_spmd
from concourse.masks import make_identity
from concourse.tile import TileContext

F32 = mybir.dt.float32
BF16 = mybir.dt.bfloat16
AF = mybir.ActivationFunctionType
ALU = mybir.AluOpType

B, S, H = 32, 4096, 512
N_CORES = 8
BPC = B // N_CORES  # examples per core

P = 128          # partitions
C = H // P       # feature chunks (4)
CH = 512         # s-chunk (matmul moving free dim)
TPC = CH // P    # s-tiles per chunk (4)


def build_nc(bpc: int, s_len: int, reps: int = 1) -> bass.Bass:
    NCH = s_len // CH    # chunks per example
    NT = s_len // P      # s-tiles per example

    nc = bacc.Bacc(None, target_bir_lowering=False)

    # Large inputs (per-core batch shard), natural layout.
    enc_d = nc.dram_tensor("encoder_outputs", [bpc, s_len, H], F32, kind="ExternalInput")
    cov_d = nc.dram_tensor("coverage_vector", [bpc, s_len, H], F32, kind="ExternalInput")
    # Host-prepped small tensors, partition-major layouts.
    dsT_d = nc.dram_tensor("dsT", [P, C, bpc], F32, kind="ExternalInput")
    Whb_d = nc.dram_tensor("Whb", [P, C, H], BF16, kind="ExternalInput")
    Wcb_d = nc.dram_tensor("Wcb", [P, C, H], BF16, kind="ExternalInput")
    Wsr_d = nc.dram_tensor("Wsr", [P, C, H], F32, kind="ExternalInput")
    bsum_d = nc.dram_tensor("bsumT", [P, C], F32, kind="ExternalInput")
    vb_d = nc.dram_tensor("vTb", [P, C], BF16, kind="ExternalInput")

    hst_d = nc.dram_tensor("h_star", [bpc, 1, H], F32, kind="ExternalOutput")
    attn_d = nc.dram_tensor("attn_dist", [bpc, s_len, 1], F32, kind="ExternalOutput")
    covn_d = nc.dram_tensor("coverage_new", [bpc, s_len, H], F32, kind="ExternalOutput")

    with TileContext(nc) as tc:
        with (
            tc.tile_pool(name="const", bufs=1) as const,
            tc.tile_pool(name="encf", bufs=3) as encf_pool,
            tc.tile_pool(name="covf", bufs=NCH + 1) as covf_pool,
            tc.tile_pool(name="encb", bufs=NCH + 1) as encb_pool,
            tc.tile_pool(name="covb", bufs=2) as covb_pool,
            tc.tile_pool(name="tpose", bufs=2) as tpose_pool,
            tc.tile_pool(name="tanh", bufs=3) as tanh_pool,
            tc.tile_pool(name="ex", bufs=2) as ex_pool,
            tc.tile_pool(name="psx", bufs=3, space="PSUM") as psx_pool,
            tc.tile_pool(name="pssc", bufs=2, space="PSUM") as pssc_pool,
            tc.tile_pool(name="pssm", bufs=2, space="PSUM") as pssm_pool,
            tc.tile_pool(name="pshs", bufs=1, space="PSUM") as pshs_pool,
        ):
            # ---------- constants ----------
            ident = const.tile([P, P], F32, tag="ident")
            make_identity(nc, ident)
            ones_row = const.tile([1, P], F32, tag="ones_row")
            nc.vector.memset(ones_row, 1.0)
            one11 = const.tile([1, 1], F32, tag="one11")
            nc.vector.memset(one11, 1.0)
            ones_col = const.tile([P, 1], F32, tag="ones_col")
            nc.vector.memset(ones_col, 1.0)

            Whb = const.tile([P, C, H], BF16, tag="Whb")
            nc.sync.dma_start(Whb, Whb_d[:])
            Wcb = const.tile([P, C, H], BF16, tag="Wcb")
            nc.sync.dma_start(Wcb, Wcb_d[:])
            Wsr = const.tile([P, C, H], F32, tag="Wsr")
            nc.sync.dma_start(Wsr, Wsr_d[:])
            bsumT = const.tile([P, C], F32, tag="bsumT")
            nc.sync.dma_start(bsumT, bsum_d[:])
            vTb = const.tile([P, C], BF16, tag="vTb")
            nc.sync.dma_start(vTb, vb_d[:])
            dsT = const.tile([P, C, bpc], F32, tag="dsT")
            nc.sync.dma_start(dsT, dsT_d[:])

            # c_b for all examples: [P, C(fout chunk), bpc]
            cbT = const.tile([P, C, bpc], F32, tag="cbT")
            for m in range(C):
                ps = pssm_pool.tile([P, bpc], F32, tag="small")
                for c in range(C):
                    nc.tensor.matmul(
                        ps,
                        lhsT=Wsr[:, c, m * P:(m + 1) * P],
                        rhs=dsT[:, c, :],
                        start=(c == 0),
                        stop=(c == C - 1),
                    )
                nc.vector.tensor_tensor(
                    cbT[:, m, :], ps,
                    bsumT[:, m:m + 1].to_broadcast((P, bpc)),
                    ALU.add,
                )

            for ex in [e for _ in range(reps) for e in range(bpc)]:
                # scores in s-major layout: partition = s % P, col = s // P
                scsm = ex_pool.tile([P, NT], F32, tag="scsm")
                enc_bf = []
                cov_f = []

                # ---------- phase 1: X = tanh(...), scores ----------
                for k in range(NCH):
                    sl = slice(k * CH, (k + 1) * CH)
                    e_f = encf_pool.tile([P, TPC, H], F32, tag="encf")
                    nc.sync.dma_start(e_f, enc_d[ex, sl, :].rearrange("(t p) f -> p t f", p=P))
                    c_f = covf_pool.tile([P, TPC, H], F32, tag="covf")
                    nc.sync.dma_start(c_f, cov_d[ex, sl, :].rearrange("(t p) f -> p t f", p=P))
                    cov_f.append(c_f)

                    e_b = encb_pool.tile([P, TPC, H], BF16, tag="encb")
                    nc.gpsimd.tensor_copy(e_b, e_f)
                    enc_bf.append(e_b)
                    c_b = covb_pool.tile([P, TPC, H], BF16, tag="covb")
                    nc.gpsimd.tensor_copy(c_b, c_f)

                    # One XBAR transpose per matrix-chunk: in [128 s, (t f)] ->
                    # out[p, q, s] = in[s, q*128+p], i.e. q = t*C + c.
                    encT_r = tpose_pool.tile([P, C * TPC, P], BF16, tag="encT")
                    covT_r = tpose_pool.tile([P, C * TPC, P], BF16, tag="covT")
                    nc.scalar.dma_start_transpose(encT_r, e_b.rearrange("p t f -> p (t f)"))
                    nc.scalar.dma_start_transpose(covT_r, c_b.rearrange("p t f -> p (t f)"))
                    # moving operand for feature chunk c: [:, c] is [128, t, s] = s-chunk
                    encT = encT_r.rearrange("p (t c) s -> p c t s", c=C)
                    covT = covT_r.rearrange("p (t c) s -> p c t s", c=C)

                    sc_ps = pssc_pool.tile([1, CH], F32, tag="scps")
                    for m in range(C):
                        x_ps = psx_pool.tile([P, CH], F32, tag="xps")
                        for c in range(C):
                            nc.tensor.matmul(
                                x_ps,
                                lhsT=Whb[:, c, m * P:(m + 1) * P],
                                rhs=encT[:, c, :],
                                start=(c == 0),
                                stop=False,
                            )
                        for c in range(C):
                            nc.tensor.matmul(
                                x_ps,
                                lhsT=Wcb[:, c, m * P:(m + 1) * P],
                                rhs=covT[:, c, :],
                                start=False,
                                stop=(c == C - 1),
                            )
                        th = tanh_pool.tile([P, CH], BF16, tag="tanh")
                        nc.scalar.activation(th, x_ps, AF.Tanh, bias=cbT[:, m, ex:ex + 1])
                        nc.tensor.matmul(
                            sc_ps,
                            lhsT=vTb[:, m:m + 1],
                            rhs=th,
                            start=(m == 0),
                            stop=(m == C - 1),
                        )
                    sc_sb = tanh_pool.tile([1, CH], F32, tag="sc_sb")
                    nc.vector.tensor_copy(sc_sb, sc_ps)
                    # transpose chunk scores into s-major columns of scsm
                    sct_ps = pssm_pool.tile([P, TPC], F32, tag="small")
                    for t in range(TPC):
                        nc.tensor.matmul(
                            sct_ps[:, t:t + 1],
                            lhsT=sc_sb[:, t * P:(t + 1) * P],
                            rhs=one11,
                            start=True,
                            stop=True,
                        )
                    nc.vector.tensor_copy(scsm[:, k * TPC:(k + 1) * TPC], sct_ps)

                # ---------- softmax over all S ----------
                m1 = ex_pool.tile([P, 1], F32, tag="m1")
                nc.vector.tensor_reduce(m1, scsm, mybir.AxisListType.X, ALU.max)
                mT_ps = pssm_pool.tile([1, P], F32, tag="small")
                nc.tensor.transpose(mT_ps, m1, ident)
                mT = ex_pool.tile([1, P], F32, tag="mT")
                nc.vector.tensor_copy(mT, mT_ps)
                m0 = ex_pool.tile([1, 1], F32, tag="m0")
                nc.vector.tensor_reduce(m0, mT, mybir.AxisListType.X, ALU.max)

                # broadcast -max to [P, 1]
                nm_ps = pssm_pool.tile([P, 1], F32, tag="small")
                nc.tensor.matmul(nm_ps, lhsT=ones_row, rhs=m0, start=True, stop=True)
                negm = ex_pool.tile([P, 1], F32, tag="negm")
                nc.vector.tensor_scalar_mul(negm, nm_ps, -1.0)

                esm = ex_pool.tile([P, NT], F32, tag="esm")
                rowsum = ex_pool.tile([P, 1], F32, tag="rowsum")
                nc.scalar.activation(esm, scsm, AF.Exp, bias=negm, accum_out=rowsum)

                tot_ps = pssm_pool.tile([1, 1], F32, tag="small")
                nc.tensor.matmul(tot_ps, lhsT=rowsum, rhs=ones_col, start=True, stop=True)
                tot = ex_pool.tile([1, 1], F32, tag="tot")
                nc.vector.tensor_copy(tot, tot_ps)
                rtot = ex_pool.tile([1, 1], F32, tag="rtot")
                nc.vector.reciprocal(rtot, tot)
                rb_ps = pssm_pool.tile([P, 1], F32, tag="small")
                nc.tensor.matmul(rb_ps, lhsT=ones_row, rhs=rtot, start=True, stop=True)
                rbc = ex_pool.tile([P, 1], F32, tag="rbc")
                nc.vector.tensor_copy(rbc, rb_ps)

                attn_sm = ex_pool.tile([P, NT], F32, tag="attn_sm")
                nc.vector.tensor_scalar_mul(attn_sm, esm, rbc)
                attn_bf = ex_pool.tile([P, NT], BF16, tag="attn_bf")
                nc.vector.tensor_copy(attn_bf, attn_sm)

                # attn output, s-contiguous [NT, P]
                at_ps = pssm_pool.tile([NT, P], F32, tag="small")
                nc.tensor.transpose(at_ps, attn_sm, ident)
                at_sb = ex_pool.tile([NT, P], F32, tag="at_sb")
                nc.vector.tensor_copy(at_sb, at_ps)
                nc.sync.dma_start(attn_d[ex, :, 0].rearrange("(t p) -> t p", p=P), at_sb)

                # ---------- phase 2: h_star, coverage_new ----------
                hs_ps = pshs_pool.tile([1, H], F32, tag="hs")
                for k in range(NCH):
                    sl = slice(k * CH, (k + 1) * CH)
                    for t in range(TPC):
                        kt = k * TPC + t
                        nc.tensor.matmul(
                            hs_ps,
                            lhsT=attn_bf[:, kt:kt + 1],
                            rhs=enc_bf[k][:, t, :],
                            start=(kt == 0),
                            stop=(kt == NT - 1),
                        )
                    for t in range(TPC):
                        kt = k * TPC + t
                        nc.vector.tensor_scalar_add(
                            cov_f[k][:, t, :], cov_f[k][:, t, :], attn_sm[:, kt:kt + 1]
                        )
                    nc.scalar.dma_start(
                        covn_d[ex, sl, :].rearrange("(t p) f -> p t f", p=P), cov_f[k]
                    )
                hst_sb = ex_pool.tile([1, H], F32, tag="hst_sb")
                nc.vector.tensor_copy(hst_sb, hs_ps)
                nc.sync.dma_start(hst_d[ex:ex + 1, 0, :], hst_sb)

    nc.compile()
    return nc


_NC_CACHE = {}


def _get_nc(bpc, s_len):
    key = (bpc, s_len)
    if key not in _NC_CACHE:
        _NC_CACHE[key] = build_nc(bpc, s_len)
    return _NC_CACHE[key]


def _prep_pmajor(W):
    """[H, H] -> [P, C, H] with partition = f_in % P, c = f_in // P."""
    return np.ascontiguousarray(W.reshape(C, P, H).transpose(1, 0, 2))


def kernel(decoder_state, encoder_outputs, coverage_vector,
           W_h, b_h, W_s, b_s, W_c, b_c, v, _trace=False, _s_len=S, _n_cores=N_CORES):
    decoder_state = np.asarray(decoder_state, dtype=np.float32)
    encoder_outputs = np.ascontiguousarray(np.asarray(encoder_outputs, dtype=np.float32))
    coverage_vector = np.ascontiguousarray(np.asarray(coverage_vector, dtype=np.float32))
    W_h = np.asarray(W_h, dtype=np.float32)
    W_s = np.asarray(W_s, dtype=np.float32)
    W_c = np.asarray(W_c, dtype=np.float32)
    b_sum = (np.asarray(b_h, dtype=np.float32) + np.asarray(b_s, dtype=np.float32)
             + np.asarray(b_c, dtype=np.float32))
    v = np.asarray(v, dtype=np.float32)

    n_cores = _n_cores
    batch = decoder_state.shape[0]
    bpc = batch // n_cores
    nc = _get_nc(bpc, _s_len)

    shared = {
        "Whb": _prep_pmajor(W_h).astype(ml_dtypes.bfloat16),
        "Wcb": _prep_pmajor(W_c).astype(ml_dtypes.bfloat16),
        "Wsr": _prep_pmajor(W_s),
        "bsumT": np.ascontiguousarray(b_sum.reshape(C, P).T),
        "vTb": np.ascontiguousarray(v.reshape(C, P).T).astype(ml_dtypes.bfloat16),
    }

    in_maps = []
    for i in range(n_cores):
        sl = slice(i * bpc, (i + 1) * bpc)
        # dsT: [P, C, bpc] feature-major decoder state for this core's shard
        dsT = np.ascontiguousarray(
            decoder_state[sl, 0, :].T.reshape(C, P, bpc).transpose(1, 0, 2)
        )
        m = {
            "encoder_outputs": encoder_outputs[sl],
            "coverage_vector": coverage_vector[sl],
            "dsT": dsT,
        }
        m.update(shared)
        in_maps.append(m)

    res = run_bass_kernel_spmd(nc, in_maps, list(range(n_cores)), trace=_trace)
    h_star = np.concatenate([r["h_star"] for r in res.results], axis=0)
    attn = np.concatenate([r["attn_dist"] for r in res.results], axis=0)
    covnew = np.concatenate([r["coverage_new"] for r in res.results], axis=0)
    kernel.last_exec_time_ns = res.exec_time_ns
    kernel.last_results = res
    return h_star, attn, covnew
